# revision 27
# baseline (speedup 1.0000x reference)
"""DCNv2 (modulated deformable conv) forward on 8 Trainium2 NeuronCores.

Problem: input [4,64,96,96], offset [4,18,96,96], mask [4,9,96,96],
weight [64,64,3,3], bias [64] -> out [4,64,96,96]. STRIDE=1, PAD=1, DIL=1,
deformable groups G=1.

Sharding: data-parallel over (batch, H-half): core = b*2 + h handles output
rows [48h, 48h+48) of batch b; weight/bias replicated.

v2: the 324 per-(tile,tap) indirect DMAs of the previous version (each
~1089ns ucode + ~310ns gap => ~455us serial floor on the Pool engine) are
replaced by 12 InstDMAGatherAnt instructions of 3456 descriptors each.
Measured SWDGE desc-gen runs ~7.7ns/desc regardless of instruction size
(Q7-pair rate; queues do NOT parallelize desc-gen - the whole Q7 cluster
is dispatched per instruction), so the gather stream floor becomes
~12*(994 + 3456*7.7) ~= 332us with instruction-issue overhead amortized.
single_packet=False is required: the default concatenated-stream mode
overflows the 16KB-per-DMA-engine packet limit and wedges the device.

dma_gather contract (verified on HW): gathered row i lands on dst
partition i%128 slot i//128; its index lives at idxs[i%16, i//16] (int16,
replicated each 16 partitions - each Q7 core pair reads its own group).
The pixel->idx-slot fold (partition j=16q+eta -> row eta, col 8*slot+q)
is done on-device: PE-transpose idxf [128,(t,k)] -> [tk,128], DVE strided
copy (f32->i16) reorders 16q+eta -> 8*eta+q, store to a DRAM scratch
[324,128], then per-chunk wrapped loads [16,S*72] (16B-contiguous runs)
replicated into 8 partition groups.

Everything else is the previous version's pipeline: quad-packed bf16 pad
tensors (one 512B read at row r = 96*fy+fx+97-base returns all four
bilinear neighbors), layout-only host prep, offset math (floor, fracs,
validity, bilinear*mask weights) on device, per-tile DVE bilinear ->
PE transpose -> fp32 matmul -> bias -> HWDGE store.
"""

import os
import sys
import types
import numpy as np

for _p in ("/opt/trn_rl_repo",):
    if _p not in sys.path and os.path.isdir(_p):
        sys.path.append(_p)

os.environ.setdefault("NEURON_RT_RESET_CORES", "1")

try:
    import antenv.axon_hooks  # noqa: F401
except ImportError:
    _hookmod = types.ModuleType("antenv.axon_hooks")
    _hookmod._hook = None
    _hookmod.set_axon_ntff_profile_hook = lambda h: setattr(_hookmod, "_hook", h)
    _hookmod.get_axon_ntff_profile_hook = lambda: _hookmod._hook
    sys.modules["antenv.axon_hooks"] = _hookmod

B, C, H, W = 4, 64, 96, 96
K = 9
Co = 64
HW = H * W                  # 9216
N_CORES = 8
HHALF = 48
NPIX = HHALF * W            # 4608 output pixels per core
NT = NPIX // 128            # 36 tiles
CHUNK_OFFS = (0, 128, 256, 384, 448)  # samp free-dim transpose chunks

PAD_ROWS = 4000             # rows per pad-half tensor (per-core global base
                            # varies and is passed via the basev input)

ACHUNKS = ((0, 1), (1, 4), (4, 9), (9, 18), (18, 27), (27, 36))
GCHUNKS = ((0, 1), (1, 4), (4, 7), (7, 9), (9, 12), (12, 15), (15, 18),
           (18, 21), (21, 24), (24, 27), (27, 30), (30, 33), (33, 36))
NGC = len(GCHUNKS)

_CACHE = {}


def _build_module():
    from contextlib import ExitStack

    import concourse.bass as bass
    import concourse.tile as tile
    from concourse import bacc, mybir, hw_specs
    from concourse.bass_interp import get_hw_module

    # Calibrate the scheduling cost model to the MEASURED InstDMAGatherAnt
    # descriptor-generation rate (~7.9ns/desc on HW vs the 0.34ns/desc
    # dense-DMA calibration).  Tile's scheduler orders each engine queue
    # from a no-exec CoreSim pass; with the default constant it models the
    # 27.4us gathers as 2.2us, so it neither feeds the gather stream's idx
    # loads promptly (late first gather) nor drains compute under it.
    hw_specs.TRN2Spec.SWDGE_NS_PER_DESCRIPTOR = 7.9

    f32 = mybir.dt.float32
    bf16 = mybir.dt.bfloat16
    i16 = mybir.dt.int16
    i32 = mybir.dt.int32
    Alu = mybir.AluOpType
    Act = mybir.ActivationFunctionType

    nc = bacc.Bacc("TRN2", target_bir_lowering=False, debug=False,
                   enable_asserts=False, num_devices=N_CORES,
                   dynamic_dma_scratch_size=32768, num_swdge_queues=4)

    omT_ap = nc.dram_tensor("omT", [128, NT * 27], f32, kind="ExternalInput").ap()
    byx_ap = nc.dram_tensor("byx", [128, NT * K * 2], f32, kind="ExternalInput").ap()
    w2_ap = nc.dram_tensor("w2", [5 * 128, Co], f32, kind="ExternalInput").ap()
    bias_ap = nc.dram_tensor("biasv", [Co, 1], f32, kind="ExternalInput").ap()
    padT = nc.dram_tensor("padtop", [PAD_ROWS, 2 * C], bf16, kind="ExternalInput")
    padB = nc.dram_tensor("padbot", [PAD_ROWS, 2 * C], bf16, kind="ExternalInput")
    basev_ap = nc.dram_tensor("basev", [128, 2], f32, kind="ExternalInput").ap()
    ident_ap = nc.dram_tensor("identv", [128, 128], f32, kind="ExternalInput").ap()
    out_ap = nc.dram_tensor("out", [Co, NPIX], f32, kind="ExternalOutput").ap()

    # overlapping quad-row views: row r = 256 bf16 at element offset 128*r
    pad_rows = (
        bass.AP(padT, 0, [[128, PAD_ROWS - 1], [1, 256]]),
        bass.AP(padB, 0, [[128, PAD_ROWS - 1], [1, 256]]),
    )

    NAC = len(ACHUNKS)      # 5 phase-A chunks (3,6,9,9,9 tiles)

    with tile.TileContext(nc) as tc:
        with ExitStack() as ctx:
            cpool = ctx.enter_context(tc.tile_pool(name="consts", bufs=1))
            prep = ctx.enter_context(tc.tile_pool(name="prep", bufs=1))
            fpsum = ctx.enter_context(tc.tile_pool(name="fold_ps", bufs=2,
                                                   space="PSUM"))
            t2pool = ctx.enter_context(tc.tile_pool(name="t2", bufs=2))
            dpool = ctx.enter_context(tc.tile_pool(name="d3", bufs=1,
                                                   space="DRAM"))
            tb_ps = ctx.enter_context(tc.tile_pool(name="trB_ps", bufs=4, space="PSUM"))
            opsum = ctx.enter_context(tc.tile_pool(name="opsum", bufs=2, space="PSUM"))

            V = nc.vector
            AD = nc.sync        # idx-pipeline DMAs share the SP queue: the
                                # Act engine's sequencer is needed for wqx /
                                # sampT / bias compute (HWDGE issue there
                                # serializes with it and costs ~85us)

            # ---- minimal loads gating the first gather ----
            omT = [prep.tile([128, (t1 - t0) * 27], f32, name=f"omTc{ci}",
                             tag=f"omTc{ci}")
                   for ci, (t0, t1) in enumerate(ACHUNKS)]
            byxc = [cpool.tile([128, (t1 - t0) * 18], f32, name=f"byxc{ci}",
                               tag=f"byxc{ci}")
                    for ci, (t0, t1) in enumerate(ACHUNKS)]
            nc.sync.dma_start(out=omT[0][:],
                              in_=omT_ap[:, 0:ACHUNKS[0][1] * 27])
            nc.sync.dma_start(out=byxc[0][:],
                              in_=byx_ap[:, 0:ACHUNKS[0][1] * 18])
            basev_sb = cpool.tile([128, 2], f32)
            nc.sync.dma_start(out=basev_sb[:], in_=basev_ap)

            # identity from host: make_identity's gpsimd AFFINE_SELECT would
            # force a mid-stream Q7 library reload right before the first
            # dma_gather (~11us stall on the Pool engine)
            ident = cpool.tile([128, 128], f32)
            nc.sync.dma_start(out=ident[:], in_=ident_ap)
            identb = cpool.tile([128, 128], bf16)
            V.tensor_copy(out=identb[:], in_=ident[:])

            # D3[p, ((t-t0)*9+k)*8+q] = idx[16q+(p%16), t, k]  (i16):
            # replicated across the 8 16-row groups at store time so each
            # gather chunk needs ONE per-partition-contiguous idx load
            D3s = [dpool.tile([128, (t1 - t0) * K * 8], i16, name=f"d3_{ci}",
                              tag=f"d3_{ci}")
                   for ci, (t0, t1) in enumerate(ACHUNKS)]

            idxf = [prep.tile([128, (t1 - t0) * K], f32, name=f"idxf{ci}",
                              tag=f"idxf{ci}")
                    for ci, (t0, t1) in enumerate(ACHUNKS)]
            wq = [prep.tile([128, (t1 - t0) * K * 4], bf16, name=f"wq{ci}",
                            tag=f"wq{ci}")
                  for ci, (t0, t1) in enumerate(ACHUNKS)]
            tmp = {}

            def a3a(ci):
                """Gather-index chain: pypx -> floor -> idx (f32, clamped).
                y and x are processed together: omT channels 0..17 are
                (dy0,dx0,dy1,dx1,...) and byx matches that (k,s) order."""
                t0, t1 = ACHUNKS[ci]
                nt = t1 - t0
                h = 0 if t0 < NT // 2 else 1
                omT3 = omT[ci][:].rearrange("p (t c) -> p t c", t=nt)
                byx18 = byxc[ci][:].rearrange("p (t x) -> p t x", t=nt)
                pypx = prep.tile([128, nt * 18], f32, name=f"pypx{ci}",
                                 tag=f"pypx{ci}")
                pypxv = pypx[:].rearrange("p (t x) -> p t x", t=nt)
                f = prep.tile([128, nt * 18], f32, name=f"f{ci}", tag=f"f{ci}")
                fv = f[:].rearrange("p (t x) -> p t x", t=nt)
                w = prep.tile([128, nt * 18], f32, name=f"w{ci}", tag=f"w{ci}")
                wv = w[:].rearrange("p (t x) -> p t x", t=nt)
                ta = prep.tile([128, nt * 18], f32, name=f"tca{ci}", tag=f"tca{ci}")
                tav = ta[:].rearrange("p (t x) -> p t x", t=nt)
                tb = prep.tile([128, nt * 18], f32, name=f"tcb{ci}", tag=f"tcb{ci}")
                tbv = tb[:].rearrange("p (t x) -> p t x", t=nt)
                ti = prep.tile([128, nt * 18], i32, name=f"ti{ci}", tag=f"ti{ci}")
                tiv = ti[:].rearrange("p (t x) -> p t x", t=nt)
                tmp[("f", ci)] = f
                tmp[("w", ci)] = w

                # pypx = d + base; floor via cast-roundtrip (any rounding mode)
                V.tensor_tensor(out=pypxv, in0=omT3[:, :, 0:18], in1=byx18,
                                op=Alu.add)
                V.tensor_copy(out=tiv, in_=pypxv)
                V.tensor_copy(out=tav, in_=tiv)
                V.tensor_tensor(out=tbv, in0=tav, in1=pypxv, op=Alu.is_gt)
                V.tensor_tensor(out=fv, in0=tav, in1=tbv, op=Alu.subtract)

                # idx = clamp(96*fy + fx + (97 - base[core,h]), 0, PAD_ROWS-2)
                f3 = f[:].rearrange("p (t k s) -> p t k s", t=nt, k=K)
                idxfv = idxf[ci][:].rearrange("p (t k) -> p t k", t=nt)
                V.scalar_tensor_tensor(out=idxfv, in0=f3[:, :, :, 0],
                                       scalar=96.0, in1=f3[:, :, :, 1],
                                       op0=Alu.mult, op1=Alu.add)
                V.tensor_scalar(out=idxf[ci][:], in0=idxf[ci][:],
                                scalar1=basev_sb[:, h:h + 1],
                                scalar2=None, op0=Alu.add)
                V.tensor_scalar(out=idxf[ci][:], in0=idxf[ci][:], scalar1=0.0,
                                scalar2=float(PAD_ROWS - 2),
                                op0=Alu.max, op1=Alu.min)

                # fractional parts (wy, wx interleaved)
                V.tensor_tensor(out=wv, in0=pypxv, in1=fv, op=Alu.subtract)

            def fold(ci):
                """idxf [128,(t,k)] -> D3 rows: transpose, 16q+eta -> 8eta+q
                free shuffle fused with the f32->i16 cast, store."""
                eng = AD
                nt = ACHUNKS[ci][1] - ACHUNKS[ci][0]
                fps = fpsum.tile([nt * K, 128], f32, name=f"fps{ci}",
                                 tag="fps")
                nc.tensor.transpose(out=fps[:], in_=idxf[ci][:],
                                    identity=ident[:])
                t2 = t2pool.tile([nt * K, 128], i16, name=f"t2_{ci}", tag="t2")
                V.tensor_copy(
                    out=t2[:].rearrange("p (e q) -> p e q", e=16),
                    in_=fps[:].rearrange("p (q e) -> p e q", q=8))
                tk = nt * K
                for g in range(8):
                    eng.dma_start(
                        out=D3s[ci][:][16 * g:16 * (g + 1), :].rearrange(
                            "e (s q) -> s e q", s=tk),
                        in_=t2[:].rearrange("s (e q) -> s e q", e=16))

            def a3b(ci):
                """Bilinear*mask weights (consumed only after gathers land)."""
                nt = ACHUNKS[ci][1] - ACHUNKS[ci][0]
                omT3 = omT[ci][:].rearrange("p (t c) -> p t c", t=nt)
                mv = omT3[:, :, 18:27]
                f = tmp[("f", ci)]
                w = tmp[("w", ci)]
                f3 = f[:].rearrange("p (t k s) -> p t k s", t=nt, k=K)
                w3 = w[:].rearrange("p (t k s) -> p t k s", t=nt, k=K)
                fyv, fxv = f3[:, :, :, 0], f3[:, :, :, 1]
                wyv, wxv = w3[:, :, :, 0], w3[:, :, :, 1]

                def t3(name):
                    t = prep.tile([128, nt * K], f32, name=f"{name}{ci}",
                                  tag=f"{name}{ci}")
                    return t, t[:].rearrange("p (t k) -> p t k", t=nt)

                ta, tav = t3("t9a")
                tb, tbv = t3("t9b")
                vm0, vm0v = t3("vm0")
                vm1, vm1v = t3("vm1")
                vc0, vc0v = t3("vc0")
                vc1, vc1v = t3("vc1")
                cA, cAv = t3("cA")
                cB, cBv = t3("cB")
                # row validity (* mask) and column validity
                V.tensor_scalar(out=tav, in0=fyv, scalar1=0.0, scalar2=None, op0=Alu.is_ge)
                V.tensor_scalar(out=tbv, in0=fyv, scalar1=95.0, scalar2=None, op0=Alu.is_le)
                V.tensor_tensor(out=vm0v, in0=tav, in1=tbv, op=Alu.mult)
                V.tensor_tensor(out=vm0v, in0=vm0v, in1=mv, op=Alu.mult)
                V.tensor_scalar(out=tav, in0=fyv, scalar1=-1.0, scalar2=None, op0=Alu.is_ge)
                V.tensor_scalar(out=tbv, in0=fyv, scalar1=94.0, scalar2=None, op0=Alu.is_le)
                V.tensor_tensor(out=vm1v, in0=tav, in1=tbv, op=Alu.mult)
                V.tensor_tensor(out=vm1v, in0=vm1v, in1=mv, op=Alu.mult)
                V.tensor_scalar(out=tav, in0=fxv, scalar1=0.0, scalar2=None, op0=Alu.is_ge)
                V.tensor_scalar(out=tbv, in0=fxv, scalar1=95.0, scalar2=None, op0=Alu.is_le)
                V.tensor_tensor(out=vc0v, in0=tav, in1=tbv, op=Alu.mult)
                V.tensor_scalar(out=tav, in0=fxv, scalar1=-1.0, scalar2=None, op0=Alu.is_ge)
                V.tensor_scalar(out=tbv, in0=fxv, scalar1=94.0, scalar2=None, op0=Alu.is_le)
                V.tensor_tensor(out=vc1v, in0=tav, in1=tbv, op=Alu.mult)

                # bilinear coefficients: cy0/cy1 (carry mask), cx0/cx1
                V.tensor_scalar(out=tav, in0=wyv, scalar1=-1.0, scalar2=1.0,
                                op0=Alu.mult, op1=Alu.add)
                V.tensor_tensor(out=cAv, in0=tav, in1=vm0v, op=Alu.mult)   # cy0
                V.tensor_tensor(out=cBv, in0=wyv, in1=vm1v, op=Alu.mult)   # cy1
                V.tensor_scalar(out=tav, in0=wxv, scalar1=-1.0, scalar2=1.0,
                                op0=Alu.mult, op1=Alu.add)
                V.tensor_tensor(out=vc0v, in0=tav, in1=vc0v, op=Alu.mult)  # cx0
                V.tensor_tensor(out=vc1v, in0=wxv, in1=vc1v, op=Alu.mult)  # cx1

                wq5 = wq[ci][:].rearrange("p (t k l v) -> p t k l v", t=nt, k=K, l=2)
                V.tensor_tensor(out=wq5[:, :, :, 0, 0], in0=cAv, in1=vc0v, op=Alu.mult)
                V.tensor_tensor(out=wq5[:, :, :, 0, 1], in0=cBv, in1=vc0v, op=Alu.mult)
                V.tensor_tensor(out=wq5[:, :, :, 1, 0], in0=cAv, in1=vc1v, op=Alu.mult)
                V.tensor_tensor(out=wq5[:, :, :, 1, 1], in0=cBv, in1=vc1v, op=Alu.mult)

            # ---- remaining constant tiles (loads emitted after the
            # first gather so they don't gate it on the SP FIFO) ----
            w2_sb = cpool.tile([128, 5 * Co], f32)
            bias_sb = cpool.tile([Co, 1], f32)

            gpool3 = ctx.enter_context(tc.tile_pool(name="gather3", bufs=5))
            gpool_s = ctx.enter_context(tc.tile_pool(name="gathers", bufs=1))
            ipool = ctx.enter_context(tc.tile_pool(name="idxw", bufs=8))
            wgpool = ctx.enter_context(tc.tile_pool(name="wg", bufs=4))
            spool = ctx.enter_context(tc.tile_pool(name="samp", bufs=4))
            stpool = ctx.enter_context(tc.tile_pool(name="sampT", bufs=4))
            obpool = ctx.enter_context(tc.tile_pool(name="ob", bufs=3))
            wqxpool = ctx.enter_context(tc.tile_pool(name="wqx", bufs=4))

            g_tiles = {}

            def emit_gather(n):
                """One SWDGE gather: (tg1-tg0) tiles x 9 taps x 128 px."""
                tg0, tg1 = GCHUNKS[n]
                h = 0 if tg0 < NT // 2 else 1
                ns = (tg1 - tg0) * K             # slots
                ni = ns * 128                    # descriptors
                idxw = ipool.tile([128, ns * 8], i16, name=f"ixw{n}",
                                  tag=f"idxw{tg1 - tg0}")
                ci = next(i for i, (t0, t1) in enumerate(ACHUNKS)
                          if t0 <= tg0 < t1)
                r0 = (tg0 - ACHUNKS[ci][0]) * K
                AD.dma_start(out=idxw[:],
                             in_=D3s[ci][:][:, r0 * 8:(r0 + ns) * 8])
                gp = gpool3 if tg1 - tg0 == 3 else gpool_s
                gt = gp.tile([128, ns * 256], bf16, name=f"g{n}",
                             tag=f"g{tg1 - tg0}")
                g_tiles[n] = gt
                nc.gpsimd.dma_gather(
                    out_ap=gt[:].rearrange("p (s e) -> p s e", s=ns),
                    in_ap=pad_rows[h],
                    idxs_ap=idxw[:],
                    num_idxs=ni,
                    num_idxs_reg=ni,
                    elem_size=256,
                    elem_step=128,
                    single_packet=False,
                    queue_num=n % 4,
                )

            def emit_compute(t):
                ci = next(i for i, (t0, t1) in enumerate(ACHUNKS)
                          if t0 <= t < t1)
                tl = t - ACHUNKS[ci][0]
                gn = next(i for i, (g0, g1) in enumerate(GCHUNKS)
                          if g0 <= t < g1)
                go = t - GCHUNKS[gn][0]
                g = g_tiles[gn][:, go * K * 4 * C:(go + 1) * K * 4 * C]
                # expand wq (k,l,v) -> (k,l,v,c) on the Scalar engine so the
                # DVE multiply below is fully contiguous bf16
                nt_c = ACHUNKS[ci][1] - ACHUNKS[ci][0]
                wq_t = wq[ci][:].rearrange(
                    "p (t k l v) -> p t k l v", t=nt_c, k=K, l=2)[:, tl]
                wq_b = wq_t.unsqueeze(4).to_broadcast([128, K, 2, 2, C])
                wqx = wqxpool.tile([128, K * 4 * C], bf16, name=f"wqx{t}",
                                   tag="wqx")
                nc.scalar.activation(
                    out=wqx[:].rearrange("p (k l v c) -> p k l v c",
                                         k=K, l=2, v=2),
                    in_=wq_b, func=Act.Copy)
                wg = wgpool.tile([128, K * 4 * C], bf16, name=f"wg{t}", tag="wg")
                V.tensor_tensor(out=wg[:], in0=g, in1=wqx[:], op=Alu.mult)

                wg5 = wg[:].rearrange("p (k l v c) -> p k l v c", k=K, l=2, v=2)
                s01 = spool.tile([128, K * 2 * C], bf16, tag="s01")
                s013 = s01[:].rearrange("p (k v c) -> p k v c", k=K, v=2)
                V.tensor_tensor(out=s013, in0=wg5[:, :, 0, :, :],
                                in1=wg5[:, :, 1, :, :], op=Alu.add)
                samp = spool.tile([128, K * C], bf16, tag="samp")
                samp3 = samp[:].rearrange("p (k c) -> p k c", k=K)
                V.tensor_tensor(out=samp3, in0=s013[:, :, 0, :],
                                in1=s013[:, :, 1, :], op=Alu.add)

                sampT = stpool.tile([128, 5 * 128], f32, name=f"sampT{t}",
                                    tag="sampT")
                for cj, off in enumerate(CHUNK_OFFS):
                    pt = tb_ps.tile([128, 128], bf16, tag="trB")
                    nc.tensor.transpose(out=pt[:], in_=samp[:, off:off + 128],
                                        identity=identb[:])
                    nc.scalar.activation(
                        out=sampT[:, cj * 128:(cj + 1) * 128], in_=pt[:],
                        func=Act.Copy)

                po = opsum.tile([Co, 128], f32, name=f"po{t}", tag="po")
                w2v = w2_sb[:].rearrange("p (f c) -> p f c", f=5)
                for cj in range(5):
                    nc.tensor.matmul(
                        out=po[:], lhsT=w2v[:, cj, :],
                        rhs=sampT[:, cj * 128:(cj + 1) * 128],
                        start=(cj == 0), stop=(cj == 4))

                ob = obpool.tile([Co, 128], f32, name=f"ob{t}", tag="ob")
                nc.scalar.activation(out=ob[:], in_=po[:], func=Act.Identity,
                                     bias=bias_sb[:, 0:1])
                nc.sync.dma_start(out=out_ap[:, t * 128:(t + 1) * 128], in_=ob[:])

            # ---- schedule: with 4-queue overlapped gathers the stream
            # is ~100us, so phase A + folds run FIRST (all idx tiles ready)
            # and the stream then dispatches 4-in-flight without feed stalls;
            # a3b (bilinear weights) and compute drain under the stream ----
            for ci in range(1, NAC):
                t0, t1 = ACHUNKS[ci]
                nc.sync.dma_start(out=omT[ci][:],
                                  in_=omT_ap[:, t0 * 27:t1 * 27])
                nc.sync.dma_start(out=byxc[ci][:],
                                  in_=byx_ap[:, t0 * 18:t1 * 18])
            for ci in range(NAC):
                a3a(ci)
                fold(ci)
            emit_gather(0)
            emit_gather(1)
            nc.sync.dma_start(
                out=w2_sb[:].rearrange("p (f c) -> p f c", f=5),
                in_=w2_ap.rearrange("(f p) c -> p f c", p=128),
            )
            nc.sync.dma_start(out=bias_sb[:], in_=bias_ap)
            for n in range(2, NGC):
                emit_gather(n)
            for ci in range(NAC):
                a3b(ci)
            for t in range(NT):
                emit_compute(t)

    nc.compile()
    nc.m = get_hw_module(nc.m)
    return nc


def _host_prep(input, offset, mask, weight, bias):
    import ml_dtypes

    f32 = np.float32
    bf16 = ml_dtypes.bfloat16
    input = np.ascontiguousarray(input, dtype=f32)
    offset = np.ascontiguousarray(offset, dtype=f32)
    mask = np.ascontiguousarray(mask, dtype=f32)
    weight = np.ascontiguousarray(weight, dtype=f32)
    bias = np.ascontiguousarray(bias, dtype=f32)

    # The split-pad dependency scheme requires sample rows to stay within
    # each half's tensor range; |offset| < 6 gives margin of >900 rows.
    amax = float(np.abs(offset).max())
    assert amax < 6.0, f"offset magnitude {amax} exceeds pad-split safety bound"

    # weight [Co, C, 3, 3] -> W2r[(t*64+c), co], chunked at CHUNK_OFFS with
    # the 448-overlap region zeroed out of chunk 4 (rows 448..511 live in
    # chunk 3).
    wr = weight.reshape(Co, C, K)                     # [co, c, t]
    W2r = np.transpose(wr, (2, 1, 0)).reshape(C * K, Co)  # [(t,c), co]
    w2 = np.zeros((5, 128, Co), dtype=f32)
    w2[0] = W2r[0:128]
    w2[1] = W2r[128:256]
    w2[2] = W2r[256:384]
    w2[3] = W2r[384:512]
    w2[4, 64:128] = W2r[512:576]
    w2 = w2.reshape(5 * 128, Co)

    biasv = bias.reshape(Co, 1)
    kyv = (np.arange(K, dtype=f32) // 3)
    kxv = (np.arange(K, dtype=f32) % 3)

    pix = np.arange(NPIX).reshape(NT, 128)
    in_maps = []
    for core in range(N_CORES):
        b, h = core // 2, core % 2
        ho0 = h * HHALF
        ho = ho0 + pix // W
        wo = pix % W
        base_y = (ho - 1)[:, :, None] + kyv[None, None, :]   # [NT, 128, K]
        base_x = (wo - 1)[:, :, None] + kxv[None, None, :]
        byx = np.stack([base_y, base_x], axis=-1)            # [NT, 128, K, 2]
        byx = np.ascontiguousarray(
            byx.transpose(1, 0, 2, 3).reshape(128, NT * K * 2), dtype=f32)
        # offset/mask, pixel-major: omT[p, t*27+j] = om[j, t*128+p]
        om = np.concatenate(
            [offset[b, :, ho0:ho0 + HHALF, :].reshape(18, NPIX),
             mask[b, :, ho0:ho0 + HHALF, :].reshape(K, NPIX)], axis=0)
        omT = np.ascontiguousarray(
            om.reshape(27, NT, 128).transpose(2, 1, 0).reshape(128, NT * 27))
        # quad-packed bf16 pads: pad[r] = [pix(base+r-97) | pix(base+r-1)],
        # zero outside the image.  Global bases per (core-half, tile-half):
        # half A covers output rows [48h, 48h+24), half B [48h+24, 48h+48).
        P = np.ascontiguousarray(input[b].reshape(C, HW).T).astype(bf16)
        bases = (0, 1536) if h == 0 else (3936, 6336)

        def build_pad(base):
            pad = np.zeros((PAD_ROWS, 2 * C), dtype=bf16)
            for col, shift in ((0, 97), (C, 1)):
                p0 = base - shift            # pixel at local row 0
                lo = max(0, -p0)             # first local row with a pixel
                hi = min(PAD_ROWS, HW - p0)  # one past last local row
                if hi > lo:
                    pad[lo:hi, col:col + C] = P[p0 + lo:p0 + hi]
            return pad

        basev = np.tile(np.array([[97 - bases[0], 97 - bases[1]]],
                                 dtype=f32), (128, 1))
        in_maps.append({
            "identv": np.eye(128, dtype=f32),
            "omT": omT,
            "byx": byx,
            "w2": w2,
            "biasv": biasv,
            "padtop": build_pad(bases[0]),
            "padbot": build_pad(bases[1]),
            "basev": basev,
        })
    return in_maps


def kernel(input, offset, mask, weight, bias):
    from concourse.bass_utils import run_bass_kernel_spmd

    if "nc" not in _CACHE:
        _CACHE["nc"] = _build_module()
    nc = _CACHE["nc"]

    in_maps = _host_prep(input, offset, mask, weight, bias)
    res = run_bass_kernel_spmd(nc, in_maps, core_ids=list(range(N_CORES)))

    out = np.empty((B, Co, H, W), dtype=np.float32)
    for core in range(N_CORES):
        b, h = core // 2, core % 2
        ho0 = h * HHALF
        out[b, :, ho0:ho0 + HHALF, :] = \
            res.results[core]["out"].reshape(Co, HHALF, W)
    return out


# revision 28
# speedup vs baseline: 1.3284x; 1.3284x over previous
"""DCNv2 (modulated deformable conv) forward on 8 Trainium2 NeuronCores.

Problem: input [4,64,96,96], offset [4,18,96,96], mask [4,9,96,96],
weight [64,64,3,3], bias [64] -> out [4,64,96,96]. STRIDE=1, PAD=1, DIL=1,
deformable groups G=1.

Sharding: data-parallel over (batch, H-half): core = b*2 + h handles output
rows [48h, 48h+48) of batch b; weight/bias replicated.

v2: the 324 per-(tile,tap) indirect DMAs of the previous version (each
~1089ns ucode + ~310ns gap => ~455us serial floor on the Pool engine) are
replaced by 12 InstDMAGatherAnt instructions of 3456 descriptors each.
Measured SWDGE desc-gen runs ~7.7ns/desc regardless of instruction size
(Q7-pair rate; queues do NOT parallelize desc-gen - the whole Q7 cluster
is dispatched per instruction), so the gather stream floor becomes
~12*(994 + 3456*7.7) ~= 332us with instruction-issue overhead amortized.
single_packet=False is required: the default concatenated-stream mode
overflows the 16KB-per-DMA-engine packet limit and wedges the device.

dma_gather contract (verified on HW): gathered row i lands on dst
partition i%128 slot i//128; its index lives at idxs[i%16, i//16] (int16,
replicated each 16 partitions - each Q7 core pair reads its own group).
The pixel->idx-slot fold (partition j=16q+eta -> row eta, col 8*slot+q)
is done on-device: PE-transpose idxf [128,(t,k)] -> [tk,128], DVE strided
copy (f32->i16) reorders 16q+eta -> 8*eta+q, store to a DRAM scratch
[324,128], then per-chunk wrapped loads [16,S*72] (16B-contiguous runs)
replicated into 8 partition groups.

Everything else is the previous version's pipeline: quad-packed bf16 pad
tensors (one 512B read at row r = 96*fy+fx+97-base returns all four
bilinear neighbors), layout-only host prep, offset math (floor, fracs,
validity, bilinear*mask weights) on device, per-tile DVE bilinear ->
PE transpose -> fp32 matmul -> bias -> HWDGE store.
"""

import os
import sys
import types
import numpy as np

for _p in ("/opt/trn_rl_repo",):
    if _p not in sys.path and os.path.isdir(_p):
        sys.path.append(_p)

os.environ.setdefault("NEURON_RT_RESET_CORES", "1")

try:
    import antenv.axon_hooks  # noqa: F401
except ImportError:
    _hookmod = types.ModuleType("antenv.axon_hooks")
    _hookmod._hook = None
    _hookmod.set_axon_ntff_profile_hook = lambda h: setattr(_hookmod, "_hook", h)
    _hookmod.get_axon_ntff_profile_hook = lambda: _hookmod._hook
    sys.modules["antenv.axon_hooks"] = _hookmod

B, C, H, W = 4, 64, 96, 96
K = 9
Co = 64
HW = H * W                  # 9216
N_CORES = 8
HHALF = 48
NPIX = HHALF * W            # 4608 output pixels per core
NT = NPIX // 128            # 36 tiles
CHUNK_OFFS = (0, 128, 256, 384, 448)  # samp free-dim transpose chunks

PAD_ROWS = 4000             # rows per pad-half tensor (per-core global base
                            # varies and is passed via the basev input)

ACHUNKS = ((0, 1), (1, 4), (4, 9), (9, 18), (18, 27), (27, 36))
GCHUNKS = ((0, 1), (1, 4), (4, 7), (7, 9), (9, 12), (12, 15), (15, 18),
           (18, 21), (21, 24), (24, 27), (27, 30), (30, 33), (33, 36))
NGC = len(GCHUNKS)

_CACHE = {}


def _build_module():
    from contextlib import ExitStack

    import concourse.bass as bass
    import concourse.tile as tile
    from concourse import bacc, mybir, hw_specs
    from concourse.bass_interp import get_hw_module

    # Calibrate the scheduling cost model to the MEASURED InstDMAGatherAnt
    # descriptor-generation rate (~7.9ns/desc on HW vs the 0.34ns/desc
    # dense-DMA calibration).  Tile's scheduler orders each engine queue
    # from a no-exec CoreSim pass; with the default constant it models the
    # 27.4us gathers as 2.2us, so it neither feeds the gather stream's idx
    # loads promptly (late first gather) nor drains compute under it.
    hw_specs.TRN2Spec.SWDGE_NS_PER_DESCRIPTOR = 7.9

    f32 = mybir.dt.float32
    bf16 = mybir.dt.bfloat16
    i16 = mybir.dt.int16
    i32 = mybir.dt.int32
    Alu = mybir.AluOpType
    Act = mybir.ActivationFunctionType

    nc = bacc.Bacc("TRN2", target_bir_lowering=False, debug=False,
                   enable_asserts=False, num_devices=N_CORES,
                   dynamic_dma_scratch_size=32768, num_swdge_queues=4)

    omT_ap = nc.dram_tensor("omT", [128, NT * 27], f32, kind="ExternalInput").ap()
    byx_ap = nc.dram_tensor("byx", [128, NT * K * 2], f32, kind="ExternalInput").ap()
    w2_ap = nc.dram_tensor("w2", [5 * 128, Co], f32, kind="ExternalInput").ap()
    bias_ap = nc.dram_tensor("biasv", [Co, 1], f32, kind="ExternalInput").ap()
    padT = nc.dram_tensor("padtop", [PAD_ROWS, 2 * C], bf16, kind="ExternalInput")
    padB = nc.dram_tensor("padbot", [PAD_ROWS, 2 * C], bf16, kind="ExternalInput")
    basev_ap = nc.dram_tensor("basev", [128, 2], f32, kind="ExternalInput").ap()
    ident_ap = nc.dram_tensor("identv", [128, 128], f32, kind="ExternalInput").ap()
    out_ap = nc.dram_tensor("out", [Co, NPIX], f32, kind="ExternalOutput").ap()

    # overlapping quad-row views: row r = 256 bf16 at element offset 128*r
    pad_rows = (
        bass.AP(padT, 0, [[128, PAD_ROWS - 1], [1, 256]]),
        bass.AP(padB, 0, [[128, PAD_ROWS - 1], [1, 256]]),
    )

    NAC = len(ACHUNKS)      # 5 phase-A chunks (3,6,9,9,9 tiles)

    with tile.TileContext(nc) as tc:
        with ExitStack() as ctx:
            cpool = ctx.enter_context(tc.tile_pool(name="consts", bufs=1))
            prep = ctx.enter_context(tc.tile_pool(name="prep", bufs=1))
            fpsum = ctx.enter_context(tc.tile_pool(name="fold_ps", bufs=2,
                                                   space="PSUM"))
            t2pool = ctx.enter_context(tc.tile_pool(name="t2", bufs=2))
            dpool = ctx.enter_context(tc.tile_pool(name="d3", bufs=1,
                                                   space="DRAM"))
            tb_ps = ctx.enter_context(tc.tile_pool(name="trB_ps", bufs=4, space="PSUM"))
            opsum = ctx.enter_context(tc.tile_pool(name="opsum", bufs=2, space="PSUM"))

            V = nc.vector
            AD = nc.sync        # idx-pipeline DMAs share the SP queue: the
                                # Act engine's sequencer is needed for wqx /
                                # sampT / bias compute (HWDGE issue there
                                # serializes with it and costs ~85us)

            # ---- minimal loads gating the first gather ----
            omT = [prep.tile([128, (t1 - t0) * 27], f32, name=f"omTc{ci}",
                             tag=f"omTc{ci}")
                   for ci, (t0, t1) in enumerate(ACHUNKS)]
            byxc = [cpool.tile([128, (t1 - t0) * 18], f32, name=f"byxc{ci}",
                               tag=f"byxc{ci}")
                    for ci, (t0, t1) in enumerate(ACHUNKS)]
            nc.sync.dma_start(out=omT[0][:],
                              in_=omT_ap[:, 0:ACHUNKS[0][1] * 27])
            nc.sync.dma_start(out=byxc[0][:],
                              in_=byx_ap[:, 0:ACHUNKS[0][1] * 18])
            basev_sb = cpool.tile([128, 2], f32)
            nc.sync.dma_start(out=basev_sb[:], in_=basev_ap)

            # identity from host: make_identity's gpsimd AFFINE_SELECT would
            # force a mid-stream Q7 library reload right before the first
            # dma_gather (~11us stall on the Pool engine)
            ident = cpool.tile([128, 128], f32)
            nc.sync.dma_start(out=ident[:], in_=ident_ap)
            identb = cpool.tile([128, 128], bf16)
            V.tensor_copy(out=identb[:], in_=ident[:])

            # D3[p, ((t-t0)*9+k)*8+q] = idx[16q+(p%16), t, k]  (i16):
            # replicated across the 8 16-row groups at store time so each
            # gather chunk needs ONE per-partition-contiguous idx load
            D3s = [dpool.tile([128, (t1 - t0) * K * 8], i16, name=f"d3_{ci}",
                              tag=f"d3_{ci}")
                   for ci, (t0, t1) in enumerate(ACHUNKS)]

            idxf = [prep.tile([128, (t1 - t0) * K], f32, name=f"idxf{ci}",
                              tag=f"idxf{ci}")
                    for ci, (t0, t1) in enumerate(ACHUNKS)]
            wq = [prep.tile([128, (t1 - t0) * K * 4], bf16, name=f"wq{ci}",
                            tag=f"wq{ci}")
                  for ci, (t0, t1) in enumerate(ACHUNKS)]
            tmp = {}

            def a3a(ci):
                """Gather-index chain: pypx -> floor -> idx (f32, clamped).
                y and x are processed together: omT channels 0..17 are
                (dy0,dx0,dy1,dx1,...) and byx matches that (k,s) order."""
                t0, t1 = ACHUNKS[ci]
                nt = t1 - t0
                h = 0 if t0 < NT // 2 else 1
                omT3 = omT[ci][:].rearrange("p (t c) -> p t c", t=nt)
                byx18 = byxc[ci][:].rearrange("p (t x) -> p t x", t=nt)
                pypx = prep.tile([128, nt * 18], f32, name=f"pypx{ci}",
                                 tag=f"pypx{ci}")
                pypxv = pypx[:].rearrange("p (t x) -> p t x", t=nt)
                f = prep.tile([128, nt * 18], f32, name=f"f{ci}", tag=f"f{ci}")
                fv = f[:].rearrange("p (t x) -> p t x", t=nt)
                w = prep.tile([128, nt * 18], f32, name=f"w{ci}", tag=f"w{ci}")
                wv = w[:].rearrange("p (t x) -> p t x", t=nt)
                ta = prep.tile([128, nt * 18], f32, name=f"tca{ci}", tag=f"tca{ci}")
                tav = ta[:].rearrange("p (t x) -> p t x", t=nt)
                tb = prep.tile([128, nt * 18], f32, name=f"tcb{ci}", tag=f"tcb{ci}")
                tbv = tb[:].rearrange("p (t x) -> p t x", t=nt)
                ti = prep.tile([128, nt * 18], i32, name=f"ti{ci}", tag=f"ti{ci}")
                tiv = ti[:].rearrange("p (t x) -> p t x", t=nt)
                tmp[("f", ci)] = f
                tmp[("w", ci)] = w

                # pypx = d + base; floor via cast-roundtrip (any rounding mode)
                V.tensor_tensor(out=pypxv, in0=omT3[:, :, 0:18], in1=byx18,
                                op=Alu.add)
                V.tensor_copy(out=tiv, in_=pypxv)
                V.tensor_copy(out=tav, in_=tiv)
                V.tensor_tensor(out=tbv, in0=tav, in1=pypxv, op=Alu.is_gt)
                V.tensor_tensor(out=fv, in0=tav, in1=tbv, op=Alu.subtract)

                # idx = clamp(96*fy + fx + (97 - base[core,h]), 0, PAD_ROWS-2)
                f3 = f[:].rearrange("p (t k s) -> p t k s", t=nt, k=K)
                idxfv = idxf[ci][:].rearrange("p (t k) -> p t k", t=nt)
                V.scalar_tensor_tensor(out=idxfv, in0=f3[:, :, :, 0],
                                       scalar=96.0, in1=f3[:, :, :, 1],
                                       op0=Alu.mult, op1=Alu.add)
                V.tensor_scalar(out=idxf[ci][:], in0=idxf[ci][:],
                                scalar1=basev_sb[:, h:h + 1],
                                scalar2=None, op0=Alu.add)
                V.tensor_scalar(out=idxf[ci][:], in0=idxf[ci][:], scalar1=0.0,
                                scalar2=float(PAD_ROWS - 2),
                                op0=Alu.max, op1=Alu.min)

                # fractional parts (wy, wx interleaved)
                V.tensor_tensor(out=wv, in0=pypxv, in1=fv, op=Alu.subtract)

            def fold(ci):
                """idxf [128,(t,k)] -> D3 rows: transpose, 16q+eta -> 8eta+q
                free shuffle fused with the f32->i16 cast, store."""
                eng = AD
                nt = ACHUNKS[ci][1] - ACHUNKS[ci][0]
                fps = fpsum.tile([nt * K, 128], f32, name=f"fps{ci}",
                                 tag="fps")
                nc.tensor.transpose(out=fps[:], in_=idxf[ci][:],
                                    identity=ident[:])
                t2 = t2pool.tile([nt * K, 128], i16, name=f"t2_{ci}", tag="t2")
                V.tensor_copy(
                    out=t2[:].rearrange("p (e q) -> p e q", e=16),
                    in_=fps[:].rearrange("p (q e) -> p e q", q=8))
                tk = nt * K
                for g in range(8):
                    eng.dma_start(
                        out=D3s[ci][:][16 * g:16 * (g + 1), :].rearrange(
                            "e (s q) -> s e q", s=tk),
                        in_=t2[:].rearrange("s (e q) -> s e q", e=16))

            def a3b(ci):
                """Bilinear*mask weights (consumed only after gathers land)."""
                nt = ACHUNKS[ci][1] - ACHUNKS[ci][0]
                omT3 = omT[ci][:].rearrange("p (t c) -> p t c", t=nt)
                mv = omT3[:, :, 18:27]
                f = tmp[("f", ci)]
                w = tmp[("w", ci)]
                f3 = f[:].rearrange("p (t k s) -> p t k s", t=nt, k=K)
                w3 = w[:].rearrange("p (t k s) -> p t k s", t=nt, k=K)
                fyv, fxv = f3[:, :, :, 0], f3[:, :, :, 1]
                wyv, wxv = w3[:, :, :, 0], w3[:, :, :, 1]

                def t3(name):
                    t = prep.tile([128, nt * K], f32, name=f"{name}{ci}",
                                  tag=f"{name}{ci}")
                    return t, t[:].rearrange("p (t k) -> p t k", t=nt)

                ta, tav = t3("t9a")
                tb, tbv = t3("t9b")
                vm0, vm0v = t3("vm0")
                vm1, vm1v = t3("vm1")
                vc0, vc0v = t3("vc0")
                vc1, vc1v = t3("vc1")
                cA, cAv = t3("cA")
                cB, cBv = t3("cB")
                # row validity (* mask) and column validity
                V.tensor_scalar(out=tav, in0=fyv, scalar1=0.0, scalar2=None, op0=Alu.is_ge)
                V.tensor_scalar(out=tbv, in0=fyv, scalar1=95.0, scalar2=None, op0=Alu.is_le)
                V.tensor_tensor(out=vm0v, in0=tav, in1=tbv, op=Alu.mult)
                V.tensor_tensor(out=vm0v, in0=vm0v, in1=mv, op=Alu.mult)
                V.tensor_scalar(out=tav, in0=fyv, scalar1=-1.0, scalar2=None, op0=Alu.is_ge)
                V.tensor_scalar(out=tbv, in0=fyv, scalar1=94.0, scalar2=None, op0=Alu.is_le)
                V.tensor_tensor(out=vm1v, in0=tav, in1=tbv, op=Alu.mult)
                V.tensor_tensor(out=vm1v, in0=vm1v, in1=mv, op=Alu.mult)
                V.tensor_scalar(out=tav, in0=fxv, scalar1=0.0, scalar2=None, op0=Alu.is_ge)
                V.tensor_scalar(out=tbv, in0=fxv, scalar1=95.0, scalar2=None, op0=Alu.is_le)
                V.tensor_tensor(out=vc0v, in0=tav, in1=tbv, op=Alu.mult)
                V.tensor_scalar(out=tav, in0=fxv, scalar1=-1.0, scalar2=None, op0=Alu.is_ge)
                V.tensor_scalar(out=tbv, in0=fxv, scalar1=94.0, scalar2=None, op0=Alu.is_le)
                V.tensor_tensor(out=vc1v, in0=tav, in1=tbv, op=Alu.mult)

                # bilinear coefficients: cy0/cy1 (carry mask), cx0/cx1
                V.tensor_scalar(out=tav, in0=wyv, scalar1=-1.0, scalar2=1.0,
                                op0=Alu.mult, op1=Alu.add)
                V.tensor_tensor(out=cAv, in0=tav, in1=vm0v, op=Alu.mult)   # cy0
                V.tensor_tensor(out=cBv, in0=wyv, in1=vm1v, op=Alu.mult)   # cy1
                V.tensor_scalar(out=tav, in0=wxv, scalar1=-1.0, scalar2=1.0,
                                op0=Alu.mult, op1=Alu.add)
                V.tensor_tensor(out=vc0v, in0=tav, in1=vc0v, op=Alu.mult)  # cx0
                V.tensor_tensor(out=vc1v, in0=wxv, in1=vc1v, op=Alu.mult)  # cx1

                wq5 = wq[ci][:].rearrange("p (t k l v) -> p t k l v", t=nt, k=K, l=2)
                V.tensor_tensor(out=wq5[:, :, :, 0, 0], in0=cAv, in1=vc0v, op=Alu.mult)
                V.tensor_tensor(out=wq5[:, :, :, 0, 1], in0=cBv, in1=vc0v, op=Alu.mult)
                V.tensor_tensor(out=wq5[:, :, :, 1, 0], in0=cAv, in1=vc1v, op=Alu.mult)
                V.tensor_tensor(out=wq5[:, :, :, 1, 1], in0=cBv, in1=vc1v, op=Alu.mult)

            # ---- remaining constant tiles (loads emitted after the
            # first gather so they don't gate it on the SP FIFO) ----
            w2_sb = cpool.tile([128, 5 * Co], f32)
            bias_sb = cpool.tile([Co, 1], f32)

            gpool3 = ctx.enter_context(tc.tile_pool(name="gather3", bufs=5))
            gpool_s = ctx.enter_context(tc.tile_pool(name="gathers", bufs=1))
            ipool = ctx.enter_context(tc.tile_pool(name="idxw", bufs=8))
            wgpool = ctx.enter_context(tc.tile_pool(name="wg", bufs=4))
            spool = ctx.enter_context(tc.tile_pool(name="samp", bufs=4))
            stpool = ctx.enter_context(tc.tile_pool(name="sampT", bufs=4))
            obpool = ctx.enter_context(tc.tile_pool(name="ob", bufs=3))
            wqxpool = ctx.enter_context(tc.tile_pool(name="wqx", bufs=4))

            g_tiles = {}

            def emit_gather(n):
                """One SWDGE gather: (tg1-tg0) tiles x 9 taps x 128 px."""
                tg0, tg1 = GCHUNKS[n]
                h = 0 if tg0 < NT // 2 else 1
                ns = (tg1 - tg0) * K             # slots
                ni = ns * 128                    # descriptors
                idxw = ipool.tile([128, ns * 8], i16, name=f"ixw{n}",
                                  tag=f"idxw{tg1 - tg0}")
                ci = next(i for i, (t0, t1) in enumerate(ACHUNKS)
                          if t0 <= tg0 < t1)
                r0 = (tg0 - ACHUNKS[ci][0]) * K
                AD.dma_start(out=idxw[:],
                             in_=D3s[ci][:][:, r0 * 8:(r0 + ns) * 8])
                gp = gpool3 if tg1 - tg0 == 3 else gpool_s
                gt = gp.tile([128, ns * 256], bf16, name=f"g{n}",
                             tag=f"g{tg1 - tg0}")
                g_tiles[n] = gt
                nc.gpsimd.dma_gather(
                    out_ap=gt[:].rearrange("p (s e) -> p s e", s=ns),
                    in_ap=pad_rows[h],
                    idxs_ap=idxw[:],
                    num_idxs=ni,
                    num_idxs_reg=ni,
                    elem_size=256,
                    elem_step=128,
                    single_packet=False,
                    queue_num=n % 4,
                )

            def emit_compute(t):
                ci = next(i for i, (t0, t1) in enumerate(ACHUNKS)
                          if t0 <= t < t1)
                tl = t - ACHUNKS[ci][0]
                gn = next(i for i, (g0, g1) in enumerate(GCHUNKS)
                          if g0 <= t < g1)
                go = t - GCHUNKS[gn][0]
                g = g_tiles[gn][:, go * K * 4 * C:(go + 1) * K * 4 * C]
                # expand wq (k,l,v) -> (k,l,v,c) on the Scalar engine so the
                # DVE multiply below is fully contiguous bf16
                nt_c = ACHUNKS[ci][1] - ACHUNKS[ci][0]
                wq_t = wq[ci][:].rearrange(
                    "p (t k l v) -> p t k l v", t=nt_c, k=K, l=2)[:, tl]
                wq_b = wq_t.unsqueeze(4).to_broadcast([128, K, 2, 2, C])
                wqx = wqxpool.tile([128, K * 4 * C], bf16, name=f"wqx{t}",
                                   tag="wqx")
                nc.scalar.activation(
                    out=wqx[:].rearrange("p (k l v c) -> p k l v c",
                                         k=K, l=2, v=2),
                    in_=wq_b, func=Act.Copy)
                wg = wgpool.tile([128, K * 4 * C], bf16, name=f"wg{t}", tag="wg")
                V.tensor_tensor(out=wg[:], in0=g, in1=wqx[:], op=Alu.mult)

                wg5 = wg[:].rearrange("p (k l v c) -> p k l v c", k=K, l=2, v=2)
                s01 = spool.tile([128, K * 2 * C], bf16, tag="s01")
                s013 = s01[:].rearrange("p (k v c) -> p k v c", k=K, v=2)
                V.tensor_tensor(out=s013, in0=wg5[:, :, 0, :, :],
                                in1=wg5[:, :, 1, :, :], op=Alu.add)
                samp = spool.tile([128, K * C], bf16, tag="samp")
                samp3 = samp[:].rearrange("p (k c) -> p k c", k=K)
                V.tensor_tensor(out=samp3, in0=s013[:, :, 0, :],
                                in1=s013[:, :, 1, :], op=Alu.add)

                sampT = stpool.tile([128, 5 * 128], f32, name=f"sampT{t}",
                                    tag="sampT")
                for cj, off in enumerate(CHUNK_OFFS):
                    pt = tb_ps.tile([128, 128], bf16, tag="trB")
                    nc.tensor.transpose(out=pt[:], in_=samp[:, off:off + 128],
                                        identity=identb[:])
                    nc.scalar.activation(
                        out=sampT[:, cj * 128:(cj + 1) * 128], in_=pt[:],
                        func=Act.Copy)

                po = opsum.tile([Co, 128], f32, name=f"po{t}", tag="po")
                w2v = w2_sb[:].rearrange("p (f c) -> p f c", f=5)
                for cj in range(5):
                    nc.tensor.matmul(
                        out=po[:], lhsT=w2v[:, cj, :],
                        rhs=sampT[:, cj * 128:(cj + 1) * 128],
                        start=(cj == 0), stop=(cj == 4))

                ob = obpool.tile([Co, 128], f32, name=f"ob{t}", tag="ob")
                nc.scalar.activation(out=ob[:], in_=po[:], func=Act.Identity,
                                     bias=bias_sb[:, 0:1])
                nc.sync.dma_start(out=out_ap[:, t * 128:(t + 1) * 128], in_=ob[:])

            # ---- schedule: tiny chunk 0 gates the first gather (~7us in);
            # the rest of the inputs load behind it; folds stay a block
            # ahead of the serial Pool gather stream ----
            a3a(0)
            fold(0)
            emit_gather(0)
            for ci in range(1, NAC):
                t0, t1 = ACHUNKS[ci]
                nc.sync.dma_start(out=omT[ci][:],
                                  in_=omT_ap[:, t0 * 27:t1 * 27])
                nc.sync.dma_start(out=byxc[ci][:],
                                  in_=byx_ap[:, t0 * 18:t1 * 18])
            nc.sync.dma_start(
                out=w2_sb[:].rearrange("p (f c) -> p f c", f=5),
                in_=w2_ap.rearrange("(f p) c -> p f c", p=128),
            )
            nc.sync.dma_start(out=bias_sb[:], in_=bias_ap)
            a3a(1)
            fold(1)
            emit_gather(1)
            a3a(2)
            fold(2)
            emit_gather(2)
            emit_gather(3)
            a3a(3)
            fold(3)
            a3b(0)
            a3b(1)
            for n in range(4, 7):
                emit_gather(n)
            a3a(4)
            fold(4)
            a3b(2)
            for n in range(7, 10):
                emit_gather(n)
            a3a(5)
            fold(5)
            a3b(3)
            a3b(4)
            a3b(5)
            for n in range(10, NGC):
                emit_gather(n)
            for t in range(NT):
                emit_compute(t)

    nc.compile()
    nc.m = get_hw_module(nc.m)
    return nc


def _host_prep(input, offset, mask, weight, bias):
    import ml_dtypes

    f32 = np.float32
    bf16 = ml_dtypes.bfloat16
    input = np.ascontiguousarray(input, dtype=f32)
    offset = np.ascontiguousarray(offset, dtype=f32)
    mask = np.ascontiguousarray(mask, dtype=f32)
    weight = np.ascontiguousarray(weight, dtype=f32)
    bias = np.ascontiguousarray(bias, dtype=f32)

    # The split-pad dependency scheme requires sample rows to stay within
    # each half's tensor range; |offset| < 6 gives margin of >900 rows.
    amax = float(np.abs(offset).max())
    assert amax < 6.0, f"offset magnitude {amax} exceeds pad-split safety bound"

    # weight [Co, C, 3, 3] -> W2r[(t*64+c), co], chunked at CHUNK_OFFS with
    # the 448-overlap region zeroed out of chunk 4 (rows 448..511 live in
    # chunk 3).
    wr = weight.reshape(Co, C, K)                     # [co, c, t]
    W2r = np.transpose(wr, (2, 1, 0)).reshape(C * K, Co)  # [(t,c), co]
    w2 = np.zeros((5, 128, Co), dtype=f32)
    w2[0] = W2r[0:128]
    w2[1] = W2r[128:256]
    w2[2] = W2r[256:384]
    w2[3] = W2r[384:512]
    w2[4, 64:128] = W2r[512:576]
    w2 = w2.reshape(5 * 128, Co)

    biasv = bias.reshape(Co, 1)
    kyv = (np.arange(K, dtype=f32) // 3)
    kxv = (np.arange(K, dtype=f32) % 3)

    pix = np.arange(NPIX).reshape(NT, 128)
    in_maps = []
    for core in range(N_CORES):
        b, h = core // 2, core % 2
        ho0 = h * HHALF
        ho = ho0 + pix // W
        wo = pix % W
        base_y = (ho - 1)[:, :, None] + kyv[None, None, :]   # [NT, 128, K]
        base_x = (wo - 1)[:, :, None] + kxv[None, None, :]
        byx = np.stack([base_y, base_x], axis=-1)            # [NT, 128, K, 2]
        byx = np.ascontiguousarray(
            byx.transpose(1, 0, 2, 3).reshape(128, NT * K * 2), dtype=f32)
        # offset/mask, pixel-major: omT[p, t*27+j] = om[j, t*128+p]
        om = np.concatenate(
            [offset[b, :, ho0:ho0 + HHALF, :].reshape(18, NPIX),
             mask[b, :, ho0:ho0 + HHALF, :].reshape(K, NPIX)], axis=0)
        omT = np.ascontiguousarray(
            om.reshape(27, NT, 128).transpose(2, 1, 0).reshape(128, NT * 27))
        # quad-packed bf16 pads: pad[r] = [pix(base+r-97) | pix(base+r-1)],
        # zero outside the image.  Global bases per (core-half, tile-half):
        # half A covers output rows [48h, 48h+24), half B [48h+24, 48h+48).
        P = np.ascontiguousarray(input[b].reshape(C, HW).T).astype(bf16)
        bases = (0, 1536) if h == 0 else (3936, 6336)

        def build_pad(base):
            pad = np.zeros((PAD_ROWS, 2 * C), dtype=bf16)
            for col, shift in ((0, 97), (C, 1)):
                p0 = base - shift            # pixel at local row 0
                lo = max(0, -p0)             # first local row with a pixel
                hi = min(PAD_ROWS, HW - p0)  # one past last local row
                if hi > lo:
                    pad[lo:hi, col:col + C] = P[p0 + lo:p0 + hi]
            return pad

        basev = np.tile(np.array([[97 - bases[0], 97 - bases[1]]],
                                 dtype=f32), (128, 1))
        in_maps.append({
            "identv": np.eye(128, dtype=f32),
            "omT": omT,
            "byx": byx,
            "w2": w2,
            "biasv": biasv,
            "padtop": build_pad(bases[0]),
            "padbot": build_pad(bases[1]),
            "basev": basev,
        })
    return in_maps


def kernel(input, offset, mask, weight, bias):
    from concourse.bass_utils import run_bass_kernel_spmd

    if "nc" not in _CACHE:
        _CACHE["nc"] = _build_module()
    nc = _CACHE["nc"]

    in_maps = _host_prep(input, offset, mask, weight, bias)
    res = run_bass_kernel_spmd(nc, in_maps, core_ids=list(range(N_CORES)))

    out = np.empty((B, Co, H, W), dtype=np.float32)
    for core in range(N_CORES):
        b, h = core // 2, core % 2
        ho0 = h * HHALF
        out[b, :, ho0:ho0 + HHALF, :] = \
            res.results[core]["out"].reshape(Co, HHALF, W)
    return out
